# revision 36
# baseline (speedup 1.0000x reference)
"""Trainium2 Bass kernel for DoubleBinaryLinear:
    y = ((x * s0) @ B.T * s2) @ A.T * s4 + bias
with x [4, 2048, 4096] fp32 and binary (+-1) B, A [4096, 4096].

Strategy
--------
The whole module is one fixed linear map:  y = x @ M + bias  with
    M = diag(s0) @ B.T @ diag(s2) @ A.T @ diag(s4)   (4096 x 4096).

Each of the 8 cores owns a 512-wide slice of the OUTPUT features:
  phase 1: build its M slice on-device,
      Mc = s0 * (B8.T @ Ap_c)  where  B8 = B as fp8e4 (+-1 exact),
                                Ap_c = (A.T * s2 * s4)[:, c-slice] (bf16)
      -> 1024 matmuls (fp8 lhsT x bf16 rhs runs at the 1-cycle/row bf16
         rate); s0 rides the PSUM->SBUF drain as a per-partition scale;
         Mc stays resident in SBUF (bf16, 4MB)
  phase 2: y[:, c-slice] = x @ Mc + bias[c-slice]  for ALL 8192 tokens
      -> 2048 matmuls, streaming xT (bf16) from HBM.

No collectives; the host concatenates the 8 output-feature slices.

This does 3072 matmuls/core (vs 8192 for the baseline's two-stage
hi/lo kernel): building M once costs 4096^3 total but the token pass
then contracts only once instead of twice. bf16 rounding of Ap/Mc/x
gives ~2.9e-3 max rel err (gate 2e-2); PSUM accumulation is fp32 exact.

Floor: 3072 x 216 ns = 664 us/core; measured ~704 us (99%-occupancy
steady state at the 216 ns issue bound; losses are phase-1 startup DMA
plus a periodic ~432 ns hardware hiccup every ~10.6 us). Shipping B as
fp8 halves the phase-1 weight stream; HWDGE queues sustain only
~72-80 GB/s each, which bounds how fast the Ap preload + Bp stream can
feed group 0.
"""

import os

import numpy as np
import ml_dtypes

import concourse.bacc as bacc
import concourse.mybir as mybir
from concourse import tile
from concourse import bass_utils

P = 128
F32 = mybir.dt.float32
BF16 = mybir.dt.bfloat16
FP8 = mybir.dt.float8e4

IN_D = 4096
MID_D = 4096
OUT_D = 4096
BATCH = 4
SEQ = 2048
N_CORES = 8
TOK = BATCH * SEQ                   # 8192 tokens, all processed by each core
OC = OUT_D // N_CORES               # 512 output features per core
TC = 512                            # token chunk = matmul moving free dim
IG = 4                              # i-tiles per PSUM group in phase 1

nI = IN_D // P                      # 32
nM = MID_D // P                     # 32
nO = OC // P                        # 4
nCh = TOK // TC                     # 16


def _build_nc():
    nc = bacc.Bacc(None, target_bir_lowering=False)
    xTd = nc.dram_tensor("xT", [IN_D, TOK], BF16, kind="ExternalInput")
    # B is pure +-1: exact in fp8e4, halving the phase-1 weight stream.
    # s0 rides the phase-1 PSUM drain (per-partition scale over i).
    Bpd = nc.dram_tensor("Bp", [MID_D, IN_D], FP8, kind="ExternalInput")
    Apd = nc.dram_tensor("Ap", [MID_D, OC], BF16, kind="ExternalInput")
    s0d = nc.dram_tensor("s0", [P, nI], F32, kind="ExternalInput")
    bid = nc.dram_tensor("bi", [P, nO], F32, kind="ExternalInput")
    yTd = nc.dram_tensor("yT", [OC, TOK], F32, kind="ExternalOutput")

    copy = mybir.ActivationFunctionType.Copy

    with tile.TileContext(nc) as tc:
        with (
            tc.tile_pool(name="consts", bufs=1) as cpool,
            tc.tile_pool(name="mc", bufs=1) as mpool,
            tc.tile_pool(name="bw", bufs=10) as bpool,
            tc.tile_pool(name="xin", bufs=2) as xpool,
            tc.tile_pool(name="yout", bufs=4) as ypool,
            tc.tile_pool(name="psum", bufs=8, space="PSUM") as pspool,
        ):
            # Ap_c resident in SBUF: 32 tiles [128, 512] bf16 = 4MB.
            # The first UPF tiles are prefetched upfront — they transfer
            # during the ~10us engine preamble before the first matmul
            # can issue anyway. The rest load lazily inside phase-1
            # group 0, interleaved with the Bp stream on the opposite
            # queue, so group-0 DMA demand stays ~225 GB/s (2 queues).
            # Ap_c resident in SBUF: 32 tiles [128, 512] bf16 = 4MB.
            # The first UPF tiles are prefetched upfront (transferring
            # during the engine preamble); the rest load lazily inside
            # phase-1 group 0, interleaved with the Bp stream on the
            # opposite queue.
            UPF = 10
            GPL = 24   # ap tiles [GPL..31] ride the gpsimd SWDGE queue
            ap_sb = [cpool.tile([P, OC], BF16, tag=f"ap{mt}",
                                name=f"ap{mt}")
                     for mt in range(nM)]
            for mt in range(UPF):
                qa = nc.scalar if (mt % 2 == 0) else nc.sync
                qa.dma_start(ap_sb[mt][:], Apd[mt * P:(mt + 1) * P, :])
            # Third DMA channel: the last 8 Ap tiles are needed ~21-27us
            # into group 0 — plenty of slack for SWDGE's per-descriptor
            # overhead — and taking them off the HWDGE queues drops
            # group-0 demand to ~68 GB/s/queue, under the ~75 sustained.
            for m2 in range(GPL, nM):
                nc.gpsimd.dma_start(ap_sb[m2][:],
                                    Apd[m2 * P:(m2 + 1) * P, :])

            s0_t = cpool.tile([P, nI], F32, tag="s0")
            nc.scalar.dma_start(s0_t[:], s0d[:, :])
            bi_t = cpool.tile([P, nO], F32, tag="bi")
            nc.sync.dma_start(bi_t[:], bid[:, :])

            # x chunk loader: 32 tiles [128, 512] bf16 = 4MB per chunk,
            # double buffered, split over the two HWDGE queues (SP + Act).
            xts = {}

            def load_xtile(ch, it):
                t = xpool.tile([P, TC], BF16, tag=f"x{it}")
                q = nc.sync if (it % 2 == 0) else nc.scalar
                q.dma_start(
                    t[:], xTd[it * P:(it + 1) * P, ch * TC:(ch + 1) * TC])
                xts.setdefault(ch, []).append(t)

            def load_chunk(ch):
                for it in range(nI):
                    load_xtile(ch, it)

            # Phase 1: Mc[it] = sum_mt (Bp tile).T @ Ap[mt],  kept in SBUF.
            # Chunk-0 x tiles are interleaved into the tail groups so the
            # prefetch hides behind phase-1 compute without delaying the
            # Bp stream.
            mc = [mpool.tile([P, OC], BF16, tag=f"mc{i}", name=f"mc{i}")
                  for i in range(nI)]
            nG = nI // IG
            for itg in range(nG):
                pss = [pspool.tile([P, OC], F32, tag="ps", name="ps")
                       for _ in range(IG)]
                for mt in range(nM):
                    wb = bpool.tile([P, IG * P], FP8, tag="wb")
                    q = nc.sync if (mt % 2 == 0) else nc.scalar
                    q.dma_start(
                        wb[:], Bpd[mt * P:(mt + 1) * P,
                                   itg * IG * P:(itg + 1) * IG * P])
                    if itg == 0 and mt + UPF < GPL:
                        m2 = mt + UPF
                        qa = nc.scalar if (m2 % 2 == 0) else nc.sync
                        qa.dma_start(ap_sb[m2][:],
                                     Apd[m2 * P:(m2 + 1) * P, :])
                    for j in range(IG):
                        nc.tensor.matmul(pss[j][:], wb[:, j * P:(j + 1) * P],
                                         ap_sb[mt][:],
                                         start=(mt == 0), stop=(mt == nM - 1))
                for j in range(IG):
                    i = itg * IG + j
                    nc.scalar.activation(mc[i][:], pss[j][:], copy,
                                         scale=s0_t[:, i:i + 1])
                if itg >= nG - 4:
                    k0 = (itg - (nG - 4)) * 8
                    for k in range(k0, k0 + 8):
                        load_xtile(0, k)

            load_chunk(1)

            # Phase 2: yT[c-slice] = Mc.T @ xT + bias
            for ch in range(nCh):
                cur = xts.pop(ch)
                for ot in range(nO):
                    ps = pspool.tile([P, TC], F32, tag="ps", name="ps")
                    for it in range(nI):
                        nc.tensor.matmul(ps[:], mc[it][:, ot * P:(ot + 1) * P],
                                         cur[it][:],
                                         start=(it == 0), stop=(it == nI - 1))
                    yt = ypool.tile([P, TC], F32, tag="yt")
                    nc.vector.tensor_scalar_add(yt[:], ps[:], bi_t[:, ot:ot + 1])
                    nc.scalar.dma_start(
                        yTd[ot * P:(ot + 1) * P, ch * TC:(ch + 1) * TC], yt[:])
                if ch + 2 < nCh:
                    load_chunk(ch + 2)

    nc.compile()
    return nc


_NC_CACHE = None


def _get_nc():
    global _NC_CACHE
    if _NC_CACHE is None:
        _NC_CACHE = _build_nc()
    return _NC_CACHE


def make_in_maps(x, scaling0, B, scaling2, A, scaling4, bias):
    x = np.asarray(x, dtype=np.float32)
    xT = np.ascontiguousarray(
        x.reshape(TOK, IN_D).T).astype(ml_dtypes.bfloat16)
    # B is +-1, exact in fp8e4; s0 is applied on-device as the phase-1
    # PSUM-drain scale (per-partition over i).
    Bp = np.asarray(B, dtype=np.float32).astype(ml_dtypes.float8_e4m3)
    Apf = (np.asarray(A, dtype=np.float32).T
           * np.asarray(scaling2, dtype=np.float32)[:, None]
           * np.asarray(scaling4, dtype=np.float32)[None, :]
           ).astype(ml_dtypes.bfloat16)
    s0 = np.ascontiguousarray(
        np.asarray(scaling0, dtype=np.float32).reshape(nI, P).T)
    bias = np.asarray(bias, dtype=np.float32)

    in_maps = []
    for c in range(N_CORES):
        in_maps.append({
            "xT": xT,
            "Bp": Bp,
            "Ap": np.ascontiguousarray(Apf[:, c * OC:(c + 1) * OC]),
            "s0": s0,
            "bi": np.ascontiguousarray(
                bias[c * OC:(c + 1) * OC].reshape(nO, P).T),
        })
    return in_maps


def kernel(x, scaling0, B, scaling2, A, scaling4, bias):
    # The profile hook isn't available in every environment; force the
    # plain execution path.
    os.environ.setdefault("BASS_NEVER_TRACE", "1")

    in_maps = make_in_maps(x, scaling0, B, scaling2, A, scaling4, bias)
    nc = _get_nc()
    res = bass_utils.run_bass_kernel_spmd(
        nc, in_maps, core_ids=list(range(N_CORES)))

    y = np.empty((TOK, OUT_D), dtype=np.float32)
    for c in range(N_CORES):
        y[:, c * OC:(c + 1) * OC] = res.results[c]["yT"].T
    return y.reshape(BATCH, SEQ, OUT_D)


# revision 37
# speedup vs baseline: 1.0169x; 1.0169x over previous
"""Trainium2 Bass kernel for DoubleBinaryLinear:
    y = ((x * s0) @ B.T * s2) @ A.T * s4 + bias
with x [4, 2048, 4096] fp32 and binary (+-1) B, A [4096, 4096].

Strategy
--------
The whole module is one fixed linear map:  y = x @ M + bias  with
    M = diag(s0) @ B.T @ diag(s2) @ A.T @ diag(s4)   (4096 x 4096).

Each of the 8 cores owns a 512-wide slice of the OUTPUT features:
  phase 1: build its M slice on-device,
      Mc = s0 * (B8.T @ Ap_c)  where  B8 = B as fp8e4 (+-1 exact),
                                Ap_c = (A.T * s2 * s4)[:, c-slice] (bf16)
      -> 1024 matmuls (fp8 lhsT x bf16 rhs runs at the 1-cycle/row bf16
         rate); s0 rides the PSUM->SBUF drain as a per-partition scale;
         Mc stays resident in SBUF (bf16, 4MB)
  phase 2: y[:, c-slice] = x @ Mc + bias[c-slice]  for ALL 8192 tokens
      -> 2048 matmuls, streaming xT (bf16) from HBM.

No collectives; the host concatenates the 8 output-feature slices.

This does 3072 matmuls/core (vs 8192 for the baseline's two-stage
hi/lo kernel): building M once costs 4096^3 total but the token pass
then contracts only once instead of twice. bf16 rounding of Ap/Mc/x
gives ~2.9e-3 max rel err (gate 2e-2); PSUM accumulation is fp32 exact.

Floor: 3072 x 216 ns = 664 us/core; measured ~704 us (99%-occupancy
steady state at the 216 ns issue bound; losses are phase-1 startup DMA
plus a periodic ~432 ns hardware hiccup every ~10.6 us). Shipping B as
fp8 halves the phase-1 weight stream; HWDGE queues sustain only
~72-80 GB/s each, which bounds how fast the Ap preload + Bp stream can
feed group 0.
"""

import os

import numpy as np
import ml_dtypes

import concourse.bacc as bacc
import concourse.mybir as mybir
from concourse import tile
from concourse import bass_utils

P = 128
F32 = mybir.dt.float32
BF16 = mybir.dt.bfloat16
FP8 = mybir.dt.float8e4

IN_D = 4096
MID_D = 4096
OUT_D = 4096
BATCH = 4
SEQ = 2048
N_CORES = 8
TOK = BATCH * SEQ                   # 8192 tokens, all processed by each core
OC = OUT_D // N_CORES               # 512 output features per core
TC = 512                            # token chunk = matmul moving free dim
IG = 4                              # i-tiles per PSUM group in phase 1

nI = IN_D // P                      # 32
nM = MID_D // P                     # 32
nO = OC // P                        # 4
nCh = TOK // TC                     # 16


def _build_nc():
    nc = bacc.Bacc(None, target_bir_lowering=False)
    xTd = nc.dram_tensor("xT", [IN_D, TOK], BF16, kind="ExternalInput")
    # B is pure +-1: exact in fp8e4, halving the phase-1 weight stream.
    # s0 rides the phase-1 PSUM drain (per-partition scale over i).
    Bpd = nc.dram_tensor("Bp", [MID_D, IN_D], FP8, kind="ExternalInput")
    Apd = nc.dram_tensor("Ap", [MID_D, OC], BF16, kind="ExternalInput")
    s0d = nc.dram_tensor("s0", [P, nI], F32, kind="ExternalInput")
    bid = nc.dram_tensor("bi", [P, nO], F32, kind="ExternalInput")
    yTd = nc.dram_tensor("yT", [OC, TOK], F32, kind="ExternalOutput")

    copy = mybir.ActivationFunctionType.Copy

    with tile.TileContext(nc) as tc:
        with (
            tc.tile_pool(name="consts", bufs=1) as cpool,
            tc.tile_pool(name="mc", bufs=1) as mpool,
            tc.tile_pool(name="bw", bufs=10) as bpool,
            tc.tile_pool(name="xin", bufs=2) as xpool,
            tc.tile_pool(name="yout", bufs=4) as ypool,
            tc.tile_pool(name="psum", bufs=8, space="PSUM") as pspool,
        ):
            # Ap_c resident in SBUF: 32 tiles [128, 512] bf16 = 4MB.
            # The first UPF tiles are prefetched upfront — they transfer
            # during the ~10us engine preamble before the first matmul
            # can issue anyway. The rest load lazily inside phase-1
            # group 0, interleaved with the Bp stream on the opposite
            # queue, so group-0 DMA demand stays ~225 GB/s (2 queues).
            # Ap_c resident in SBUF: 32 tiles [128, 512] bf16 = 4MB.
            # The first UPF tiles are prefetched upfront (transferring
            # during the engine preamble); the rest load lazily inside
            # phase-1 group 0, interleaved with the Bp stream on the
            # opposite queue.
            UPF = 10
            ap_sb = [cpool.tile([P, OC], BF16, tag=f"ap{mt}",
                                name=f"ap{mt}")
                     for mt in range(nM)]
            for mt in range(UPF):
                qa = nc.scalar if (mt % 2 == 0) else nc.sync
                qa.dma_start(ap_sb[mt][:], Apd[mt * P:(mt + 1) * P, :])

            s0_t = cpool.tile([P, nI], F32, tag="s0")
            nc.scalar.dma_start(s0_t[:], s0d[:, :])
            bi_t = cpool.tile([P, nO], F32, tag="bi")
            nc.sync.dma_start(bi_t[:], bid[:, :])

            # x chunk loader: 32 tiles [128, 512] bf16 = 4MB per chunk,
            # double buffered, split over the two HWDGE queues (SP + Act).
            xts = {}

            def load_xtile(ch, it):
                t = xpool.tile([P, TC], BF16, tag=f"x{it}")
                q = nc.sync if (it % 2 == 0) else nc.scalar
                q.dma_start(
                    t[:], xTd[it * P:(it + 1) * P, ch * TC:(ch + 1) * TC])
                xts.setdefault(ch, []).append(t)

            def load_chunk(ch):
                for it in range(nI):
                    load_xtile(ch, it)

            # Phase 1: Mc[it] = sum_mt (Bp tile).T @ Ap[mt],  kept in SBUF.
            # Chunk-0 x tiles are interleaved into the tail groups so the
            # prefetch hides behind phase-1 compute without delaying the
            # Bp stream.
            mc = [mpool.tile([P, OC], BF16, tag=f"mc{i}", name=f"mc{i}")
                  for i in range(nI)]
            nG = nI // IG
            for itg in range(nG):
                pss = [pspool.tile([P, OC], F32, tag="ps", name="ps")
                       for _ in range(IG)]
                for mt in range(nM):
                    wb = bpool.tile([P, IG * P], FP8, tag="wb")
                    q = nc.sync if (mt % 2 == 0) else nc.scalar
                    q.dma_start(
                        wb[:], Bpd[mt * P:(mt + 1) * P,
                                   itg * IG * P:(itg + 1) * IG * P])
                    if itg == 0 and mt + UPF < nM:
                        m2 = mt + UPF
                        qa = nc.scalar if (m2 % 2 == 0) else nc.sync
                        qa.dma_start(ap_sb[m2][:],
                                     Apd[m2 * P:(m2 + 1) * P, :])
                    for j in range(IG):
                        nc.tensor.matmul(pss[j][:], wb[:, j * P:(j + 1) * P],
                                         ap_sb[mt][:],
                                         start=(mt == 0), stop=(mt == nM - 1))
                for j in range(IG):
                    i = itg * IG + j
                    nc.scalar.activation(mc[i][:], pss[j][:], copy,
                                         scale=s0_t[:, i:i + 1])
                if itg >= nG - 4:
                    k0 = (itg - (nG - 4)) * 8
                    for k in range(k0, k0 + 8):
                        load_xtile(0, k)

            load_chunk(1)

            # Phase 2: yT[c-slice] = Mc.T @ xT + bias
            for ch in range(nCh):
                cur = xts.pop(ch)
                for ot in range(nO):
                    ps = pspool.tile([P, TC], F32, tag="ps", name="ps")
                    for it in range(nI):
                        nc.tensor.matmul(ps[:], mc[it][:, ot * P:(ot + 1) * P],
                                         cur[it][:],
                                         start=(it == 0), stop=(it == nI - 1))
                    yt = ypool.tile([P, TC], F32, tag="yt")
                    nc.vector.tensor_scalar_add(yt[:], ps[:], bi_t[:, ot:ot + 1])
                    nc.scalar.dma_start(
                        yTd[ot * P:(ot + 1) * P, ch * TC:(ch + 1) * TC], yt[:])
                if ch + 2 < nCh:
                    load_chunk(ch + 2)

    nc.compile()
    return nc


_NC_CACHE = None


def _get_nc():
    global _NC_CACHE
    if _NC_CACHE is None:
        _NC_CACHE = _build_nc()
    return _NC_CACHE


def make_in_maps(x, scaling0, B, scaling2, A, scaling4, bias):
    x = np.asarray(x, dtype=np.float32)
    xT = np.ascontiguousarray(
        x.reshape(TOK, IN_D).T).astype(ml_dtypes.bfloat16)
    # B is +-1, exact in fp8e4; s0 is applied on-device as the phase-1
    # PSUM-drain scale (per-partition over i).
    Bp = np.asarray(B, dtype=np.float32).astype(ml_dtypes.float8_e4m3)
    Apf = (np.asarray(A, dtype=np.float32).T
           * np.asarray(scaling2, dtype=np.float32)[:, None]
           * np.asarray(scaling4, dtype=np.float32)[None, :]
           ).astype(ml_dtypes.bfloat16)
    s0 = np.ascontiguousarray(
        np.asarray(scaling0, dtype=np.float32).reshape(nI, P).T)
    bias = np.asarray(bias, dtype=np.float32)

    in_maps = []
    for c in range(N_CORES):
        in_maps.append({
            "xT": xT,
            "Bp": Bp,
            "Ap": np.ascontiguousarray(Apf[:, c * OC:(c + 1) * OC]),
            "s0": s0,
            "bi": np.ascontiguousarray(
                bias[c * OC:(c + 1) * OC].reshape(nO, P).T),
        })
    return in_maps


def kernel(x, scaling0, B, scaling2, A, scaling4, bias):
    # The profile hook isn't available in every environment; force the
    # plain execution path.
    os.environ.setdefault("BASS_NEVER_TRACE", "1")

    in_maps = make_in_maps(x, scaling0, B, scaling2, A, scaling4, bias)
    nc = _get_nc()
    res = bass_utils.run_bass_kernel_spmd(
        nc, in_maps, core_ids=list(range(N_CORES)))

    y = np.empty((TOK, OUT_D), dtype=np.float32)
    for c in range(N_CORES):
        y[:, c * OC:(c + 1) * OC] = res.results[c]["yT"].T
    return y.reshape(BATCH, SEQ, OUT_D)
